# revision 5
# baseline (speedup 1.0000x reference)
import numpy as np

B, L, D = 4, 4096, 512
H, DK = 8, 64
R = 64              # DFT radix (L = R*R)
MA = 25
PAD = (MA - 1) // 2  # 12
EPS = 1e-5
HALF = 2048
SLICE = 2072        # 2048 + 24 halo (clamped at sequence edges)
SOFF = 2024         # slice start for half 1
TOPK = 8
PAIRS = [[0, 1], [2, 3], [4, 5], [6, 7]]

_FN = None
_PERCORE = None


def _consts():
    i = np.arange(R)
    C64 = np.cos(2 * np.pi * np.outer(i, i) / R).astype(np.float32)
    S64 = np.sin(2 * np.pi * np.outer(i, i) / R).astype(np.float32)
    e, j = np.meshgrid(i, i, indexing="ij")
    TwC = np.cos(2 * np.pi * e * j / L).astype(np.float32)   # [e, j]
    TwS = np.sin(2 * np.pi * e * j / L).astype(np.float32)
    return C64, S64, TwC, TwS


def _build():
    global _FN, _PERCORE
    if _FN is not None:
        return _FN
    import jax
    import jax.numpy as jnp
    from jax import lax

    C64_, S64_, TwC_, TwS_ = _consts()

    def percore(x_b, x_s, r0v, Wq4, bq4, Wk4, bk4, Wv4, bv4,
                Wo, bo, W1, b1, W2, b2, g1, be1, g2, be2):
        # x_b [L,D] full batch row; x_s [SLICE,D] this core's halo'd token slice;
        # r0v [] int32 slice start; W*4 [D,4,DK] this core's 4 heads.
        C64, S64, TwC, TwS = map(jnp.asarray, (C64_, S64_, TwC_, TwS_))

        Q = jnp.einsum("ld,dhk->hlk", x_b, Wq4) + bq4.T[:, None, :]   # [4,L,dk]
        K = jnp.einsum("ld,dhk->hlk", x_b, Wk4) + bk4.T[:, None, :]
        V = jnp.einsum("ld,dhk->hlk", x_b, Wv4) + bv4.T[:, None, :]

        def fwd(X):  # [4, L, dk] -> Z(re,im) [4, e, g, dk]
            M = X.reshape(H // 2, R, R, DK)                       # t = 64 i + j
            Yre = jnp.einsum("ie,hijd->hejd", C64, M)
            Yim = -jnp.einsum("ie,hijd->hejd", S64, M)
            Tc = TwC[None, :, :, None]
            Ts = TwS[None, :, :, None]
            Ypre = Yre * Tc + Yim * Ts
            Ypim = Yim * Tc - Yre * Ts
            Zre = jnp.einsum("hejd,jg->hegd", Ypre, C64) + jnp.einsum("hejd,jg->hegd", Ypim, S64)
            Zim = jnp.einsum("hejd,jg->hegd", Ypim, C64) - jnp.einsum("hejd,jg->hegd", Ypre, S64)
            return Zre, Zim

        Qre, Qim = fwd(Q)
        Kre, Kim = fwd(K)
        Sre = jnp.einsum("hegd,hegd->heg", Qre, Kre) + jnp.einsum("hegd,hegd->heg", Qim, Kim)
        Sim = jnp.einsum("hegd,hegd->heg", Qim, Kre) - jnp.einsum("hegd,hegd->heg", Qre, Kim)

        Ure = jnp.einsum("heg,ga->hea", Sre, C64) - jnp.einsum("heg,ga->hea", Sim, S64)
        Uim = jnp.einsum("heg,ga->hea", Sim, C64) + jnp.einsum("heg,ga->hea", Sre, S64)
        Upre = Ure * TwC[None] - Uim * TwS[None]
        Upim = Uim * TwC[None] + Ure * TwS[None]
        Cm = jnp.einsum("eb,hea->hba", C64, Upre) - jnp.einsum("eb,hea->hba", S64, Upim)
        corr = Cm.reshape(H // 2, L) * (1.0 / (L * DK))           # tau = 64 b + a

        # top-8 (descending) + softmax
        ar = jnp.arange(L, dtype=jnp.int32)
        c = corr
        vals, idxs = [], []
        for _ in range(TOPK):
            m = jnp.max(c, axis=1)
            im = jnp.argmax(c, axis=1).astype(jnp.int32)
            vals.append(m)
            idxs.append(im)
            c = jnp.where(ar[None, :] == im[:, None], -1e30, c)
        tw = jax.nn.softmax(jnp.stack(vals, 1), axis=-1)          # [4, 8]
        ti = jnp.stack(idxs, 1)

        # delay aggregation: out[h] = sum_k tw[h,k] * roll(V[h], -ti[h,k])
        Vd = jnp.concatenate([V, V], axis=1)                      # [4, 2L, dk]
        outs = []
        for h in range(H // 2):
            acc = jnp.zeros((L, DK), jnp.float32)
            for k in range(TOPK):
                sl = lax.dynamic_slice(Vd[h], (ti[h, k], 0), (L, DK))
                acc = acc + tw[h, k] * sl
            outs.append(acc)
        ctx4 = jnp.stack(outs, 1)                                 # [L, 4, dk]

        g = lax.all_gather(ctx4, "c", axis_index_groups=PAIRS)    # [2, L, 4, dk]
        ctx = jnp.transpose(g, (1, 0, 2, 3)).reshape(L, D)        # head order 0..7
        ctx_s = lax.dynamic_slice(ctx, (r0v, 0), (SLICE, D))
        attn = ctx_s @ Wo + bo

        def ln(z, gg, bb):
            mu = jnp.mean(z, -1, keepdims=True)
            var = jnp.mean((z - mu) ** 2, -1, keepdims=True)
            return (z - mu) * lax.rsqrt(var + EPS) * gg + bb

        def decomp(z):
            zp = jnp.concatenate(
                [jnp.repeat(z[:1], PAD, 0), z, jnp.repeat(z[-1:], PAD, 0)], 0)
            t = zp[0:SLICE]
            for o in range(1, MA):
                t = t + zp[o:o + SLICE]
            t = t * (1.0 / MA)
            return z - t, t

        x1 = ln(x_s + attn, g1, be1)
        s1, t1 = decomp(x1)
        hmid = jax.nn.gelu(s1 @ W1 + b1, approximate=False)
        ff = hmid @ W2 + b2
        x2 = ln(s1 + ff, g2, be2)
        s2, t2 = decomp(x2)
        return s2, t1 + t2                                        # [SLICE, D] each

    _PERCORE = percore
    _FN = jax.pmap(percore, axis_name="c")
    return _FN


def _prep(x, Wq, bq, Wk, bk, Wv, bv, Wo, bo, W1, b1, W2, b2, g1, be1, g2, be2):
    f32 = lambda w: np.asarray(w, np.float32)
    x = f32(x)
    xs, xss, r0s = [], [], []
    Wstk = {n: [] for n in ("Wq4", "bq4", "Wk4", "bk4", "Wv4", "bv4")}
    for c in range(8):
        b, half = c // 2, c % 2
        r0 = half * SOFF
        xs.append(x[b])
        xss.append(x[b, r0:r0 + SLICE])
        r0s.append(r0)
        sl = slice(4 * half * DK, (4 * half + 4) * DK)
        Wstk["Wq4"].append(f32(Wq)[:, sl].reshape(D, 4, DK))
        Wstk["bq4"].append(f32(bq)[sl].reshape(4, DK).T)
        Wstk["Wk4"].append(f32(Wk)[:, sl].reshape(D, 4, DK))
        Wstk["bk4"].append(f32(bk)[sl].reshape(4, DK).T)
        Wstk["Wv4"].append(f32(Wv)[:, sl].reshape(D, 4, DK))
        Wstk["bv4"].append(f32(bv)[sl].reshape(4, DK).T)
    rep = lambda w: np.broadcast_to(f32(w), (8,) + np.shape(w))
    args = [np.stack(xs), np.stack(xss), np.asarray(r0s, np.int32)]
    args += [np.stack(Wstk[n]) for n in ("Wq4", "bq4", "Wk4", "bk4", "Wv4", "bv4")]
    args += [rep(w) for w in (Wo, bo, W1, b1, W2, b2, g1, be1, g2, be2)]
    return args


def kernel(x, Wq, bq, Wk, bk, Wv, bv, Wo, bo, W1, b1, W2, b2, g1, be1, g2, be2):
    fn = _build()
    args = _prep(x, Wq, bq, Wk, bk, Wv, bv, Wo, bo, W1, b1, W2, b2,
                 g1, be1, g2, be2)
    s2, tr = fn(*args)
    s2 = np.asarray(s2)
    tr = np.asarray(tr)
    out_s = np.empty((B, L, D), np.float32)
    out_t = np.empty((B, L, D), np.float32)
    for c in range(8):
        b, half = c // 2, c % 2
        off = half * (SLICE - HALF)
        out_s[b, half * HALF:(half + 1) * HALF] = s2[c, off:off + HALF]
        out_t[b, half * HALF:(half + 1) * HALF] = tr[c, off:off + HALF]
    return out_s, out_t
